# revision 2
# baseline (speedup 1.0000x reference)
"""Cross-attention Trainium2 kernel (8 NeuronCores, SPMD over Q rows).

Math:  out = softmax((m1 Wq^T + bq)(m2 Wk^T + bk)^T / sqrt(H)) (m2 Wv^T + bv)

v2 structure: all small matmuls are folded on the host so the device does
only the two N1*N2*E-sized matmuls per core:
    host:  GT = scale * Wq^T Wk ;  C = GT^T @ m1T  [E, N1] ; V = m2 @ Wv^T
    dev:   ST  = m2 @ C_c                  [N2, QCc]   (scores^T, per-core q slice)
           EST = exp(ST - 5)               (f16, softmax-shift is exact: row-const)
           s   = EST^T @ ones              (N=1 matmuls, one PSUM bank all kernel)
           O   = EST^T @ V                 [QCc, H]  (accumulated over k blocks)
           out = O * (1/s)

The bk term of the scores is softmax-invariant (per-q constant) and dropped
exactly; a nonzero bq adds a per-k bias handled via the ACT bias input.
Each core handles N1/8 = 1024 query rows; zero collectives.
"""

import numpy as np

E = 1024
H = 1024
N1 = 8192
N2 = 8192
NCORES = 8
SCALE = 1.0 / np.sqrt(np.float32(H))
SHIFT = 5.0   # softmax-invariant; keeps exp in fp16 range

_BUILD_CACHE = {}


def _build(biased_q=False, e=E, h=H, n2=N2, qc=N1 // NCORES, kb=1024,
           repeat=1, st_bufs=3, av_bufs=2, sum_mode="pe1", stage="full"):
    """Build (and finalize) the per-core Bass kernel. Returns nc."""
    import concourse.bacc as bacc
    import concourse.tile as tile
    import concourse.mybir as mybir
    import contextlib

    f32 = mybir.dt.float32
    f16 = mybir.dt.float16

    et = e // 128          # e tiles (contraction for ST)
    qt = qc // 128         # q tiles
    ktb = kb // 128        # k tiles per block
    nb = n2 // kb          # k blocks
    qw = 512               # q chunk width for ST (matmul N)
    hw_ = 512              # h chunk width for AV
    qch = qc // qw
    hch = h // hw_
    EXP = mybir.ActivationFunctionType.Exp

    nc = bacc.Bacc(None, target_bir_lowering=False)

    c_d = nc.dram_tensor("c", [e, qc], f16, kind="ExternalInput")
    m2t_d = nc.dram_tensor("m2t", [e, n2], f16, kind="ExternalInput")
    v_d = nc.dram_tensor("v", [n2, h], f16, kind="ExternalInput")
    d2_d = nc.dram_tensor("d2", [128, n2 // 128], f32, kind="ExternalInput") if biased_q else None
    out_d = nc.dram_tensor("out", [qc, h], f32, kind="ExternalOutput")

    with tile.TileContext(nc) as tc:
        rep_ctx = tc.For_i(0, repeat, 1) if repeat > 1 else contextlib.nullcontext()
        with rep_ctx, tc.tile_pool(name="res", bufs=1) as res:
            shift_sb = res.tile([128, 1], f32)
            nc.vector.memset(shift_sb, -SHIFT)
            ones = res.tile([128, 1], f16)
            zeros = res.tile([128, 128], f16)
            onesf = res.tile([128, 1], f32)
            nc.vector.memset(onesf, 1.0)
            nc.vector.tensor_copy(ones, onesf)
            nc.vector.memset(zeros, 0.0)
            z8 = res.tile([128, 8], f16)
            nc.vector.memset(z8, 0.0)

            c_sb = res.tile([128, et, qc], f16)
            for t in range(et):
                nc.sync.dma_start(out=c_sb[:, t, :], in_=c_d[t * 128:(t + 1) * 128, :])
            if biased_q:
                d2_sb = res.tile([128, n2 // 128], f32)
                nc.sync.dma_start(out=d2_sb, in_=d2_d[:, :])
            out_sb = res.tile([128, qt, h], f32)

            with (
                tc.tile_pool(name="m2tp", bufs=2) as m2tp,
                tc.tile_pool(name="vp", bufs=2) as vp,
                tc.tile_pool(name="estp", bufs=2) as estp,
                tc.tile_pool(name="stps", bufs=st_bufs, space="PSUM") as stps,
                tc.tile_pool(name="avps", bufs=av_bufs, space="PSUM") as avps,
                tc.tile_pool(name="sps", bufs=1, space="PSUM") as sps,
            ):
                if sum_mode == "pe1":
                    s_ps = sps.tile([128, qt], f32)
                    # zero-init the s bank; every later s-matmul accumulates
                    nc.tensor.matmul(s_ps, zeros, z8[:, 0:qt],
                                     start=True, stop=False, skip_group_check=True)
                else:
                    acc = res.tile([128, qc], f32)
                for b in range(nb):
                    k0 = b * kb
                    m2t_blk = m2tp.tile([128, et, kb], f16, tag="m2t")
                    for t in range(et):
                        nc.sync.dma_start(
                            out=m2t_blk[:, t, :],
                            in_=m2t_d[t * 128:(t + 1) * 128, k0:k0 + kb])
                    v_blk = vp.tile([128, ktb, h], f16, tag="v")
                    for j in range(ktb):
                        nc.sync.dma_start(
                            out=v_blk[:, j, :],
                            in_=v_d[k0 + j * 128:k0 + (j + 1) * 128, :])
                    est = estp.tile([128, ktb, qc], f16, tag="est")

                    # scores^T for this block (+ optional d2 bias), then exp
                    for j in range(ktb):
                        for q0 in range(qch):
                            stp = stps.tile([128, qw], f32, tag="st")
                            for t in range(et):
                                nc.tensor.matmul(
                                    stp,
                                    m2t_blk[:, t, j * 128:(j + 1) * 128],
                                    c_sb[:, t, q0 * qw:(q0 + 1) * qw],
                                    start=(t == 0), stop=(t == et - 1),
                                )
                            nc.scalar.activation(
                                est[:, j, q0 * qw:(q0 + 1) * qw], stp, EXP,
                                bias=(d2_sb[:, (k0 // 128) + j:(k0 // 128) + j + 1]
                                      if biased_q else shift_sb), scale=1.0)
                    if stage == "st":
                        continue

                    # O += EST^T @ V  and  s += EST^T @ ones
                    for t in range(qt):
                        av = avps.tile([128, h], f32, tag="av")
                        for h0 in range(hch):
                            for j in range(ktb):
                                nc.tensor.matmul(
                                    av[:, h0 * hw_:(h0 + 1) * hw_],
                                    est[:, j, t * 128:(t + 1) * 128],
                                    v_blk[:, j, h0 * hw_:(h0 + 1) * hw_],
                                    start=(j == 0), stop=(j == ktb - 1),
                                )
                        if sum_mode == "pe1":
                            for j in range(ktb):
                                nc.tensor.matmul(
                                    s_ps[:, t:t + 1],
                                    est[:, j, t * 128:(t + 1) * 128],
                                    ones,
                                    start=False,
                                    stop=(b == nb - 1 and t == qt - 1 and j == ktb - 1),
                                    skip_group_check=True,
                                )
                        dst = out_sb[:, t, :]
                        if b == 0:
                            nc.vector.tensor_copy(dst, av)
                        else:
                            nc.vector.tensor_add(dst, dst, av)
                    if sum_mode == "dve":
                        for j in range(ktb):
                            if b == 0 and j == 0:
                                nc.vector.tensor_copy(acc, est[:, 0, :])
                            else:
                                nc.vector.tensor_add(acc, acc, est[:, j, :])

                # ---- normalize + store ----
                if stage == "full":
                    with (
                        tc.tile_pool(name="epi", bufs=1) as epi,
                        tc.tile_pool(name="ob", bufs=3) as obp,
                    ):
                        recip = epi.tile([128, qt], f32)
                        if sum_mode == "pe1":
                            nc.vector.reciprocal(recip, s_ps)
                        else:
                            with tc.tile_pool(name="eps", bufs=1, space="PSUM") as eps:
                                scp = eps.tile([128, qt], f32)
                                for q0 in range(qt):
                                    nc.tensor.matmul(
                                        scp[:, q0:q0 + 1], acc[:, q0 * 128:(q0 + 1) * 128],
                                        onesf, start=(q0 == 0), stop=(q0 == qt - 1),
                                        skip_group_check=True)
                                nc.vector.reciprocal(recip, scp)
                        for t in range(qt):
                            ob = obp.tile([128, h], f32, tag="ob")
                            nc.vector.tensor_scalar_mul(ob, out_sb[:, t, :], recip[:, t:t + 1])
                            nc.sync.dma_start(out=out_d[t * 128:(t + 1) * 128, :], in_=ob)
                elif stage == "nosum":
                    for t in range(qt):
                        nc.sync.dma_start(out=out_d[t * 128:(t + 1) * 128, :],
                                          in_=out_sb[:, t, :])

    nc.finalize()
    return nc


def _get_nc(key):
    if key not in _BUILD_CACHE:
        _BUILD_CACHE[key] = _build(**dict(key))
    return _BUILD_CACHE[key]


def _prep_inputs(molecule1, molecule2, Wq, bq, Wk, bk, Wv, bv):
    """Host-side prep. Returns (in_maps, biased_q)."""
    m1 = np.asarray(molecule1, np.float32)
    m2 = np.ascontiguousarray(np.asarray(molecule2, np.float32))
    wq = np.asarray(Wq, np.float64)
    wk = np.asarray(Wk, np.float64)
    wv = np.asarray(Wv, np.float32)
    bq64 = np.asarray(bq, np.float64)
    bv32 = np.asarray(bv, np.float32)
    assert not np.any(bv32), "nonzero bv not supported by v2 kernel"

    scale = 1.0 / np.sqrt(np.float64(wq.shape[0]))
    gt = (scale * (wq.T @ wk)).astype(np.float32)      # [E, E]
    cx = m1 @ gt                                        # [N1, E] f32
    m2t = np.ascontiguousarray(m2.T).astype(np.float16)  # [E, N2]
    v = (m2 @ wv.T).astype(np.float16)                   # [N2, H]

    d2 = (scale * (m2 @ (wk.T @ bq64))).astype(np.float32)  # [N2] per-k bias
    biased_q = bool(np.any(d2))

    qc = m1.shape[0] // NCORES
    in_maps = []
    for c in range(NCORES):
        m = {
            "c": np.ascontiguousarray(cx[c * qc:(c + 1) * qc, :].T).astype(np.float16),
            "m2t": m2t,
            "v": v,
        }
        if biased_q:
            # d2 laid out [128 partitions, n2/128 k-tiles], with bias folded
            # together with the -SHIFT softmax shift
            m["d2"] = np.ascontiguousarray(
                (d2 - SHIFT).reshape(-1, 128).T).astype(np.float32)
        in_maps.append(m)
    return in_maps, biased_q


def kernel(molecule1, molecule2, Wq, bq, Wk, bk, Wv, bv):
    from concourse.bass_utils import run_bass_kernel_spmd

    in_maps, biased_q = _prep_inputs(
        molecule1, molecule2, Wq, bq, Wk, bk, Wv, bv)
    key = (("biased_q", biased_q),)
    nc = _get_nc(key)
    res = run_bass_kernel_spmd(nc, in_maps, core_ids=list(range(NCORES)))
    out = np.concatenate([res.results[c]["out"] for c in range(NCORES)], axis=0)
    return out.astype(np.asarray(molecule1).dtype, copy=False)


# revision 16
# speedup vs baseline: 1.0034x; 1.0034x over previous
"""Cross-attention Trainium2 kernel (8 NeuronCores, SPMD over Q rows).

Math:  out = softmax((m1 Wq^T + bq)(m2 Wk^T + bk)^T / sqrt(H)) (m2 Wv^T + bv)

v2 structure: all small matmuls are folded on the host so the device does
only the two N1*N2*E-sized matmuls per core:
    host:  GT = scale * Wq^T Wk ;  C = GT^T @ m1T  [E, N1] ; V = m2 @ Wv^T
    dev:   ST  = m2 @ C_c                  [N2, QCc]   (scores^T, per-core q slice)
           EST = exp(ST - 5)               (f16, softmax-shift is exact: row-const)
           s   = EST^T @ ones              (N=1 matmuls, one PSUM bank all kernel)
           O   = EST^T @ V                 [QCc, H]  (accumulated over k blocks)
           out = O * (1/s)

The bk term of the scores is softmax-invariant (per-q constant) and dropped
exactly; a nonzero bq adds a per-k bias handled via the ACT bias input.
Each core handles N1/8 = 1024 query rows; zero collectives.
"""

import numpy as np

E = 1024
H = 1024
N1 = 8192
N2 = 8192
NCORES = 8
SCALE = 1.0 / np.sqrt(np.float32(H))
SHIFT = 5.0   # softmax-invariant; keeps exp in fp16 range

_BUILD_CACHE = {}


def _build(biased_q=False, e=E, h=H, n2=N2, qc=N1 // NCORES, kb=512,
           repeat=1, st_bufs=4, av_bufs=2, av_banks=1, sum_mode="pe1",
           ilv=True, est_bufs=2, in_bufs=2, st_reuse=True, stage="full"):
    """Build (and finalize) the per-core Bass kernel. Returns nc."""
    import concourse.bacc as bacc
    import concourse.tile as tile
    import concourse.mybir as mybir
    import contextlib

    f32 = mybir.dt.float32
    f16 = mybir.dt.float16

    et = e // 128          # e tiles (contraction for ST)
    qt = qc // 128         # q tiles
    ktb = kb // 128        # k tiles per block
    nb = n2 // kb          # k blocks
    qw = 512               # q chunk width for ST (matmul N)
    hw_ = 512              # h chunk width for AV
    qch = qc // qw
    hch = h // hw_
    EXP = mybir.ActivationFunctionType.Exp

    nc = bacc.Bacc(None, target_bir_lowering=False)

    c_d = nc.dram_tensor("c", [e, qc], f16, kind="ExternalInput")
    m2t_d = nc.dram_tensor("m2t", [e, n2], f16, kind="ExternalInput")
    v_d = nc.dram_tensor("v", [n2, h], f16, kind="ExternalInput")
    d2_d = nc.dram_tensor("d2", [128, n2 // 128], f32, kind="ExternalInput") if biased_q else None
    out_d = nc.dram_tensor("out", [qc, h], f32, kind="ExternalOutput")

    with tile.TileContext(nc) as tc:
        rep_ctx = tc.For_i(0, repeat, 1) if repeat > 1 else contextlib.nullcontext()
        with rep_ctx, tc.tile_pool(name="res", bufs=1) as res:
            shift_sb = res.tile([128, 1], f32)
            nc.vector.memset(shift_sb, -SHIFT)
            ones = res.tile([128, 1], f16)
            zeros = res.tile([128, 128], f16)
            onesf = res.tile([128, 1], f32)
            nc.vector.memset(onesf, 1.0)
            nc.vector.tensor_copy(ones, onesf)
            nc.vector.memset(zeros, 0.0)
            z8 = res.tile([128, 8], f16)
            nc.vector.memset(z8, 0.0)

            c_sb = res.tile([128, et, qc], f16)
            for t in range(et):
                nc.sync.dma_start(out=c_sb[:, t, :], in_=c_d[t * 128:(t + 1) * 128, :])
            if biased_q:
                d2_sb = res.tile([128, n2 // 128], f32)
                nc.sync.dma_start(out=d2_sb, in_=d2_d[:, :])
            out_sb = res.tile([128, qt, h], f32)

            with (
                tc.tile_pool(name="m2tp", bufs=in_bufs) as m2tp,
                tc.tile_pool(name="vp", bufs=in_bufs) as vp,
                tc.tile_pool(name="estp", bufs=est_bufs) as estp,
                tc.tile_pool(name="stps", bufs=st_bufs, space="PSUM") as stps,
                tc.tile_pool(name="avps", bufs=av_bufs, space="PSUM") as avps,
                tc.tile_pool(name="sps", bufs=1, space="PSUM") as sps,
            ):
                if sum_mode == "pe1" and stage != "st":
                    s_ps = sps.tile([128, qt], f32)
                    # zero-init the s bank; every later s-matmul accumulates
                    nc.tensor.matmul(s_ps, zeros, z8[:, 0:qt],
                                     start=True, stop=False, skip_group_check=True)
                else:
                    acc = res.tile([128, qc], f32)
                for b in range(nb):
                    k0 = b * kb
                    m2t_blk = m2tp.tile([128, et, kb], f16, tag="m2t")
                    for t in range(et):
                        nc.sync.dma_start(
                            out=m2t_blk[:, t, :],
                            in_=m2t_d[t * 128:(t + 1) * 128, k0:k0 + kb])
                    v_blk = vp.tile([128, ktb, h], f16, tag="v")
                    for j in range(ktb):
                        nc.sync.dma_start(
                            out=v_blk[:, j, :],
                            in_=v_d[k0 + j * 128:k0 + (j + 1) * 128, :])
                    est = estp.tile([128, ktb, qc], f16, tag="est")

                    # scores^T for this block (+ optional d2 bias), then exp
                    for j in range(ktb):
                        if st_reuse:
                            # one stationary m2t tile serves both q-chunks:
                            # qch accumulation groups proceed interleaved
                            stq = [stps.tile([128, qw], f32, tag="st",
                                             name=f"stq{q0}")
                                   for q0 in range(qch)]
                            for t in range(et):
                                for q0 in range(qch):
                                    nc.tensor.matmul(
                                        stq[q0],
                                        m2t_blk[:, t, j * 128:(j + 1) * 128],
                                        c_sb[:, t, q0 * qw:(q0 + 1) * qw],
                                        start=(t == 0), stop=(t == et - 1),
                                    )
                            for q0 in range(qch):
                                nc.scalar.activation(
                                    est[:, j, q0 * qw:(q0 + 1) * qw], stq[q0], EXP,
                                    bias=(d2_sb[:, (k0 // 128) + j:(k0 // 128) + j + 1]
                                          if biased_q else shift_sb), scale=1.0)
                        else:
                            for q0 in range(qch):
                                stp = stps.tile([128, qw], f32, tag="st")
                                for t in range(et):
                                    nc.tensor.matmul(
                                        stp,
                                        m2t_blk[:, t, j * 128:(j + 1) * 128],
                                        c_sb[:, t, q0 * qw:(q0 + 1) * qw],
                                        start=(t == 0), stop=(t == et - 1),
                                    )
                                nc.scalar.activation(
                                    est[:, j, q0 * qw:(q0 + 1) * qw], stp, EXP,
                                    bias=(d2_sb[:, (k0 // 128) + j:(k0 // 128) + j + 1]
                                          if biased_q else shift_sb), scale=1.0)
                    if stage == "st":
                        continue

                    # O += EST^T @ V  and  s += EST^T @ ones
                    for t in range(qt):
                        if av_banks == 2:
                            av0 = avps.tile([128, h], f32, tag="av")
                            avs = [av0]
                            avc = [av0[:, h0 * hw_:(h0 + 1) * hw_] for h0 in range(hch)]
                        else:
                            av0 = avps.tile([128, hw_], f32, tag="av")
                            av1 = avps.tile([128, hw_], f32, tag="av")
                            avs = [av0, av1]
                            avc = avs
                        def s_mm(j):
                            nc.tensor.matmul(
                                s_ps[:, t:t + 1],
                                est[:, j, t * 128:(t + 1) * 128],
                                ones,
                                start=False,
                                stop=(b == nb - 1 and j == ktb - 1),
                                skip_group_check=True,
                            )
                        if ilv:
                            # one stationary est tile feeds hch+1 matmuls; the
                            # N=1 sum-matmul's LDW hides under the wide streams
                            for j in range(ktb):
                                for h0 in range(hch):
                                    nc.tensor.matmul(
                                        avc[h0],
                                        est[:, j, t * 128:(t + 1) * 128],
                                        v_blk[:, j, h0 * hw_:(h0 + 1) * hw_],
                                        start=(j == 0), stop=(j == ktb - 1),
                                    )
                                if sum_mode == "pe1":
                                    s_mm(j)
                        else:
                            for h0 in range(hch):
                                for j in range(ktb):
                                    nc.tensor.matmul(
                                        avc[h0],
                                        est[:, j, t * 128:(t + 1) * 128],
                                        v_blk[:, j, h0 * hw_:(h0 + 1) * hw_],
                                        start=(j == 0), stop=(j == ktb - 1),
                                    )
                            if sum_mode == "pe1":
                                for j in range(ktb):
                                    s_mm(j)
                        if av_banks == 2:
                            dst = out_sb[:, t, :]
                            if b == 0:
                                nc.vector.tensor_copy(dst, avs[0])
                            else:
                                nc.vector.tensor_add(dst, dst, avs[0])
                        else:
                            for h0 in range(hch):
                                dst = out_sb[:, t, h0 * hw_:(h0 + 1) * hw_]
                                if b == 0:
                                    nc.vector.tensor_copy(dst, avs[h0])
                                else:
                                    nc.vector.tensor_add(dst, dst, avs[h0])
                    if sum_mode == "dve":
                        for j in range(ktb):
                            if b == 0 and j == 0:
                                nc.vector.tensor_copy(acc, est[:, 0, :])
                            else:
                                nc.vector.tensor_add(acc, acc, est[:, j, :])

                # ---- normalize + store ----
                if stage == "full":
                    with (
                        tc.tile_pool(name="epi", bufs=1) as epi,
                        tc.tile_pool(name="ob", bufs=3) as obp,
                    ):
                        recip = epi.tile([128, qt], f32)
                        if sum_mode == "pe1":
                            # per-column: tile t's normalize can start as soon
                            # as its own sums and av-adds land
                            for t in range(qt):
                                nc.vector.reciprocal(recip[:, t:t + 1], s_ps[:, t:t + 1])
                        elif sum_mode == "dve":
                            with tc.tile_pool(name="eps", bufs=1, space="PSUM") as eps:
                                scp = eps.tile([128, qt], f32)
                                for q0 in range(qt):
                                    nc.tensor.matmul(
                                        scp[:, q0:q0 + 1], acc[:, q0 * 128:(q0 + 1) * 128],
                                        onesf, start=(q0 == 0), stop=(q0 == qt - 1),
                                        skip_group_check=True)
                                nc.vector.reciprocal(recip, scp)
                        for t in range(qt):
                            ob = obp.tile([128, h], f32, tag="ob")
                            nc.vector.tensor_scalar_mul(ob, out_sb[:, t, :], recip[:, t:t + 1])
                            nc.sync.dma_start(out=out_d[t * 128:(t + 1) * 128, :], in_=ob)
                elif stage == "nosum":
                    for t in range(qt):
                        nc.sync.dma_start(out=out_d[t * 128:(t + 1) * 128, :],
                                          in_=out_sb[:, t, :])

    nc.finalize()
    return nc


def _get_nc(key):
    if key not in _BUILD_CACHE:
        _BUILD_CACHE[key] = _build(**dict(key))
    return _BUILD_CACHE[key]


def _prep_inputs(molecule1, molecule2, Wq, bq, Wk, bk, Wv, bv, stagger=None):
    """Host-side prep. Returns (in_maps, biased_q).

    stagger: rotate each core's k axis by a different offset so the 8 cores'
    DMA streams and phase transitions decorrelate (softmax sums and the AV
    accumulation are k-order invariant, so this is exact)."""
    import os
    if stagger is None:
        stagger = os.environ.get("STAGGER", "1") == "1"
    m1 = np.asarray(molecule1, np.float32)
    m2 = np.ascontiguousarray(np.asarray(molecule2, np.float32))
    wq = np.asarray(Wq, np.float64)
    wk = np.asarray(Wk, np.float64)
    wv = np.asarray(Wv, np.float32)
    bq64 = np.asarray(bq, np.float64)
    bv32 = np.asarray(bv, np.float32)
    assert not np.any(bv32), "nonzero bv not supported by v2 kernel"

    scale = 1.0 / np.sqrt(np.float64(wq.shape[0]))
    gt = (scale * (wq.T @ wk)).astype(np.float32)      # [E, E]
    cx = m1 @ gt                                        # [N1, E] f32
    m2t = np.ascontiguousarray(m2.T).astype(np.float16)  # [E, N2]
    v = (m2 @ wv.T).astype(np.float16)                   # [N2, H]

    d2 = (scale * (m2 @ (wk.T @ bq64))).astype(np.float32)  # [N2] per-k bias
    biased_q = bool(np.any(d2))
    n2 = m2.shape[0]

    qc = m1.shape[0] // NCORES
    in_maps = []
    for c in range(NCORES):
        off = (c * n2) // NCORES if stagger else 0
        m = {
            "c": np.ascontiguousarray(cx[c * qc:(c + 1) * qc, :].T).astype(np.float16),
            "m2t": np.ascontiguousarray(np.roll(m2t, -off, axis=1)) if off else m2t,
            "v": np.ascontiguousarray(np.roll(v, -off, axis=0)) if off else v,
        }
        if biased_q:
            # d2 laid out [128 partitions, n2/128 k-tiles], with bias folded
            # together with the -SHIFT softmax shift
            m["d2"] = np.ascontiguousarray(
                (np.roll(d2, -off) - SHIFT).reshape(-1, 128).T).astype(np.float32)
        in_maps.append(m)
    return in_maps, biased_q


def kernel(molecule1, molecule2, Wq, bq, Wk, bk, Wv, bv):
    from concourse.bass_utils import run_bass_kernel_spmd

    in_maps, biased_q = _prep_inputs(
        molecule1, molecule2, Wq, bq, Wk, bk, Wv, bv)
    key = (("biased_q", biased_q),)
    nc = _get_nc(key)
    res = run_bass_kernel_spmd(nc, in_maps, core_ids=list(range(NCORES)))
    out = np.concatenate([res.results[c]["out"] for c in range(NCORES)], axis=0)
    return out.astype(np.asarray(molecule1).dtype, copy=False)
